# revision 3
# baseline (speedup 1.0000x reference)
"""Trainium2 Bass kernel for nn_CPSN (retrieval_knn PSM/PWG module).

Contract: kernel(**inputs) takes the FULL unsharded inputs (as produced by
setup_inputs) and returns the FULL output [2, b*q, s], distributing work
across 8 NeuronCores internally (data-parallel over the query dim q).

Algorithm per (q, s) pair (b=1, s=25, q=30, c=512, hw=361):
  O[x, y] = <f2n[:, x], f1n[:, y]>   (x = query pixel, y = support pixel)
  s21[x] = max_y O ; s12[y] = max_x O
  g1[x] = a1[argmax_y O[x, :]] ; g2[y] = a2[argmax_x O[:, y]]
  w = g1 * g2 ; out0 = mean(s12 * w) ; out1 = mean(s21 * w)

Design notes:
  - L2 normalization and the meta-learner run on host; the device receives
    pre-normalized bf16 features (bf16 matmul = 1 PE cycle/row vs fp32's 4).
  - bf16 halves SBUF/DMA: the full f1n fits resident, no ss-blocking.
  - Each PSUM tile is staged to SBUF as bf16 by the Activation engine; the
    DVE reduce_max and the fused argmax-gather
    (scalar_tensor_tensor is_ge/mult with accum) read the staged bf16 tile,
    unlocking the DVE 4x perf mode for the gather pass.
"""

import os
import sys

import numpy as np

for _p in ("/opt/trn_rl_repo", "/root/.axon_site/_ro/trn_rl_repo"):
    if os.path.isdir(_p) and _p not in sys.path:
        sys.path.insert(0, _p)

import concourse.bass as bass
import concourse.tile as tile
from concourse import bacc, library_config, mybir
from concourse.bass_utils import run_bass_kernel_spmd

# ---- problem constants (hardcoded per contract) ----
B, S, Q, C, H, W, TEMP = 1, 25, 30, 512, 19, 19, 64
HW = H * W  # 361
NCORES = 8
L = 4               # local (padded) query images per core; Q_PAD = 32
Q_PAD = NCORES * L
CCH = C // 128      # 4 contraction chunks
PCH = [(0, 128), (128, 128), (256, HW - 256)]  # pixel-dim partition chunks
GRP = 4             # O-phase ss group size (PSUM bank budget)
BN_EPS = 1e-5

F32 = mybir.dt.float32
HDT = mybir.dt.float16
AX_X = mybir.AxisListType.X
OP = mybir.AluOpType
AF = mybir.ActivationFunctionType
HDTNP = np.float16


def _col_off(l, kind, pch, ss):
    # cols2d free layout: [L][kind:4][pchunk:3][S]
    return ((l * 4 + kind) * 3 + pch) * S + ss


def build_program(variant="", repeat=1, mmdt=None):
    """Build the (SPMD-shared) single-core bass program.

    mmdt: dtype for the feature tiles / matmul inputs (default fp16).
    """
    mmdt = mmdt or HDT
    nc = bacc.Bacc(None, target_bir_lowering=False, debug=False)

    f1_d = nc.dram_tensor("f1n", [S, C, HW], mmdt, kind="ExternalInput")
    f2_d = nc.dram_tensor("f2s", [L, C, HW], mmdt, kind="ExternalInput")
    # attention rows, host-gathered per (l, ss); broadcast to 128 rows on-device
    a1r_d = nc.dram_tensor("a1r", [L, S, HW], HDT, kind="ExternalInput")
    a2r_d = nc.dram_tensor("a2r", [L, HW], HDT, kind="ExternalInput")
    out_d = nc.dram_tensor("out", [2 * L, S], F32, kind="ExternalOutput")

    with tile.TileContext(nc) as tc:
        from contextlib import ExitStack

        with ExitStack() as ctx:
            pp = ctx.enter_context(tc.tile_pool(name="pp", bufs=2, space="PSUM"))
            nsep = CCH if "sep" in variant else 1
            f1n_pool = ctx.enter_context(
                tc.tile_pool(name="f1n", bufs=S * nsep))
            f2n_pool = ctx.enter_context(
                tc.tile_pool(name="f2n", bufs=L * nsep))
            a1bc_pool = ctx.enter_context(tc.tile_pool(name="a1bc", bufs=2 * S))
            a2bc_pool = ctx.enter_context(tc.tile_pool(name="a2bc", bufs=L))
            stg_pool = ctx.enter_context(tc.tile_pool(name="stg", bufs=4))
            half_pool = ctx.enter_context(tc.tile_pool(name="half", bufs=4))
            scr_pool = ctx.enter_context(tc.tile_pool(name="scr", bufs=2))
            cols_pool = ctx.enter_context(tc.tile_pool(name="cols", bufs=1))
            cst_pool = ctx.enter_context(tc.tile_pool(name="cst", bufs=2))
            fin_pool = ctx.enter_context(tc.tile_pool(name="fin", bufs=6))

            usepool = "pool" in variant and "nopool" not in variant
            if usepool:
                nc.gpsimd.load_library(library_config.lib)

            mcol = cst_pool.tile([128, 1], F32, tag="cst")
            nc.vector.memset(mcol[:], 1.0 / HW)

            def group_max(stg, np_, ng, maxout):
                """max over the last axis of stg[0:np_, 0:ng, :] -> maxout
                cols [np_, ng].  Pool pre-halves 361 -> 181 -> 91 (overlap at
                the seam is harmless for max), DVE finishes."""
                if not usepool:
                    nc.vector.reduce_max(maxout, stg[0:np_, 0:ng, :],
                                         axis=AX_X)
                    return
                h1 = half_pool.tile([128, L, 181], HDT, name="h1", tag="h1")
                nc.gpsimd.tensor_tensor(
                    h1[0:np_, 0:ng, :], stg[0:np_, 0:ng, 0:181],
                    stg[0:np_, 0:ng, 180:361], op=OP.max)
                h2 = half_pool.tile([128, L, 91], HDT, name="h2", tag="h2")
                nc.gpsimd.tensor_tensor(
                    h2[0:np_, 0:ng, :], h1[0:np_, 0:ng, 0:91],
                    h1[0:np_, 0:ng, 90:181], op=OP.max)
                nc.vector.reduce_max(maxout, h2[0:np_, 0:ng, :], axis=AX_X)

            cols2d = cols_pool.tile([128, L * 4 * 3 * S], F32)
            cols12 = cols_pool.tile([128, 3 * S * L], F32)
            if variant:
                nc.vector.memset(cols2d[:], 1.0)
                nc.vector.memset(cols12[:], 1.0)

            # ---- load all features (bf16, fully resident); one DMA per
            # ---- image: [C, HW] dram -> [128, CCH, HW] sbuf (c in free dim).
            # ---- f2/attention rows first so phase-1 consumers aren't queued
            # ---- behind the 25 f1 image DMAs.
            def load_image(dram_t, img_idx, name, pool, tag):
                if "sep" in variant:
                    # baseline-style: one [128, HW] tile per c-chunk
                    ts = []
                    for c in range(CCH):
                        t = pool.tile([128, HW], mmdt, name=f"{name}_{c}",
                                      tag=tag)
                        nc.sync.dma_start(
                            t[:],
                            dram_t[img_idx, c * 128:(c + 1) * 128, :])
                        ts.append(t)
                    return ts
                t = pool.tile([128, CCH, HW], mmdt, name=name, tag=tag)
                src = bass.AP(dram_t.ap().tensor, img_idx * C * HW,
                              [[HW, 128], [128 * HW, CCH], [1, HW]])
                nc.sync.dma_start(t[:], src)
                return t

            def mm_ops(t, c, f0, fn_):
                # stationary/moving operand slice for contraction chunk c
                if "sep" in variant:
                    return t[c][:, f0:f0 + fn_]
                return t[:, c, f0:f0 + fn_]

            f2nt = [load_image(f2_d, l, f"f2n_{l}", f2n_pool, "f2n")
                    for l in range(L)]

            def bcast_row(dram_t, row_off, name, pool, tag):
                # replicate one DRAM row into all 128 partitions (stride-0
                # partition dim on the DMA source)
                t = pool.tile([128, HW], HDT, name=f"bc_{name}", tag=tag)
                src = bass.AP(dram_t.ap().tensor, row_off * HW,
                              [[0, 128], [1, HW]])
                nc.sync.dma_start(t[:], src)
                return t

            # a2 broadcast tiles (persist whole kernel)
            a2bc = [bcast_row(a2r_d, l, f"a2_{l}", a2bc_pool, "a2bc")
                    for l in range(L)]

            f1nt = [load_image(f1_d, ss, f"f1n_{ss}", f1n_pool, "f1n")
                    for ss in range(S)]

            for _rep in range(repeat):
                # ---- T phase: T[y, x] per (l, ss); weights = f1n chunks ----
                for ss in range(S):
                    for pi, (y0, yp) in enumerate(PCH):
                        psT = pp.tile([yp, L, 512], F32, name="psT", tag="ps")
                        if "nomm" not in variant:
                            for c in range(CCH):
                                for l in range(L):
                                    nc.tensor.matmul(
                                        psT[:, l, 0:HW],
                                        mm_ops(f1nt[ss], c, y0, yp),
                                        mm_ops(f2nt[l], c, 0, HW),
                                        start=(c == 0), stop=(c == CCH - 1))
                        else:
                            nc.vector.memset(psT[:, :, 0:1], 0.1)
                        if "nodve" in variant:
                            continue
                        stg = stg_pool.tile([128, L, HW], HDT, name="stgT",
                                            tag="stg")
                        nc.scalar.activation(
                            stg[0:yp, :, :], psT[:, :, 0:HW], AF.Copy)
                        o12 = (pi * S + ss) * L
                        group_max(stg, yp, L, cols12[0:yp, o12:o12 + L])
                        for l in range(L):
                            og = _col_off(l, 3, pi, ss)
                            scr = scr_pool.tile([128, HW], HDT, name="sttscr",
                                                tag="scr")
                            nc.vector.scalar_tensor_tensor(
                                scr[0:yp, :],
                                stg[0:yp, l, :],
                                cols12[0:yp, o12 + l:o12 + l + 1],
                                a2bc[l][0:yp, :],
                                op0=OP.is_ge, op1=OP.mult,
                                accum_out=cols2d[0:yp, og:og + 1])

                # ---- O phase: O[x, y] per (l, ss); weights = f2n chunks ----
                for l in range(L):
                    a1t = {}
                    for ss in range(S):
                        if "nobc" in variant:
                            a1t[ss] = a2bc[l]
                        else:
                            a1t[ss] = bcast_row(a1r_d, l * S + ss,
                                                f"a1_{l}_{ss}", a1bc_pool,
                                                "a1bc")
                    for pi, (x0, xp) in enumerate(PCH):
                        for g0 in range(0, S, GRP):
                            grp = list(range(g0, min(g0 + GRP, S)))
                            ng = len(grp)
                            psO = pp.tile([xp, L, 512], F32, name="psO",
                                          tag="ps")
                            if "nomm" not in variant:
                                for c in range(CCH):
                                    for j, ss in enumerate(grp):
                                        nc.tensor.matmul(
                                            psO[:, j, 0:HW],
                                            mm_ops(f2nt[l], c, x0, xp),
                                            mm_ops(f1nt[ss], c, 0, HW),
                                            start=(c == 0), stop=(c == CCH - 1))
                            else:
                                nc.vector.memset(psO[:, :, 0:1], 0.1)
                            if "nodve" in variant:
                                continue
                            stg = stg_pool.tile([128, L, HW], HDT,
                                                name="stgO", tag="stg")
                            nc.scalar.activation(
                                stg[0:xp, 0:ng, :],
                                psO[:, 0:ng, 0:HW], AF.Copy)
                            # s21 for the ng consecutive ss: contiguous cols
                            ob = _col_off(l, 0, pi, grp[0])
                            group_max(stg, xp, ng, cols2d[0:xp, ob:ob + ng])
                            for j, ss in enumerate(grp):
                                og = _col_off(l, 2, pi, ss)
                                scr = scr_pool.tile([128, HW], HDT,
                                                    name="sttscr", tag="scr")
                                nc.vector.scalar_tensor_tensor(
                                    scr[0:xp, :],
                                    stg[0:xp, j, :],
                                    cols2d[0:xp, ob + j:ob + j + 1],
                                    a1t[ss][0:xp, :],
                                    op0=OP.is_ge, op1=OP.mult,
                                    accum_out=cols2d[0:xp, og:og + 1])

                # ---- finals: w = g1*g2; out0 = mean(s12*w); out1 = mean(s21*w)
                for l in range(L):
                    fp1 = pp.tile([1, S], F32, tag="ps")
                    fp2 = pp.tile([1, S], F32, tag="ps")
                    for pi, (p0, pn) in enumerate(PCH):
                        g1 = cols2d[0:pn,
                                    _col_off(l, 2, pi, 0):_col_off(l, 2, pi, 0) + S]
                        g2 = cols2d[0:pn,
                                    _col_off(l, 3, pi, 0):_col_off(l, 3, pi, 0) + S]
                        s21 = cols2d[0:pn,
                                     _col_off(l, 0, pi, 0):_col_off(l, 0, pi, 0) + S]
                        c12 = cols12[0:pn, :]
                        s12 = bass.AP(c12.tensor, c12.offset + pi * S * L + l,
                                      [c12.ap[0], [L, S]])
                        wt = fin_pool.tile([128, S], F32, tag="fin")
                        v1 = fin_pool.tile([128, S], F32, tag="fin")
                        v2 = fin_pool.tile([128, S], F32, tag="fin")
                        nc.vector.tensor_mul(wt[0:pn, :], g1, g2)
                        nc.vector.tensor_mul(v1[0:pn, :], s12, wt[0:pn, :])
                        nc.vector.tensor_mul(v2[0:pn, :], s21, wt[0:pn, :])
                        nc.tensor.matmul(fp1[:, :], mcol[0:pn, 0:1],
                                         v1[0:pn, :],
                                         start=(pi == 0), stop=(pi == 2))
                        nc.tensor.matmul(fp2[:, :], mcol[0:pn, 0:1],
                                         v2[0:pn, :],
                                         start=(pi == 0), stop=(pi == 2))
                    st1 = fin_pool.tile([1, S], F32, name=f"st1_{l}",
                                        tag="finst")
                    st2 = fin_pool.tile([1, S], F32, name=f"st2_{l}",
                                        tag="finst")
                    nc.scalar.activation(st1[:], fp1[0:1, :], AF.Copy)
                    nc.scalar.activation(st2[:], fp2[0:1, :], AF.Copy)
                    nc.sync.dma_start(out_d[l:l + 1, :], st1[0:1, :])
                    nc.sync.dma_start(out_d[L + l:L + l + 1, :], st2[0:1, :])

    nc.finalize()
    return nc


def _meta_learner_host(x, W1, g1, b1, m1, v1, W2, g2, b2, m2, v2):
    """x: [N, C, HW] -> [N, HW]  (two 1x1 convs + eval BN + ReLU on host)."""
    inv1 = g1 / np.sqrt(v1 + BN_EPS)
    bias1 = b1 - m1 * inv1
    y = np.einsum("tc,ncp->ntp", W1, x, dtype=np.float32)
    y = np.maximum(y * inv1[None, :, None] + bias1[None, :, None], 0.0)
    inv2 = g2 / np.sqrt(v2 + BN_EPS)
    bias2 = b2 - m2 * inv2
    z = np.einsum("ot,ntp->nop", W2, y, dtype=np.float32)
    z = np.maximum(z * inv2[None, :, None] + bias2[None, :, None], 0.0)
    return z[:, 0, :]


def _l2n(x):
    n = np.sqrt(np.einsum("ncp,ncp->np", x, x, dtype=np.float32))
    return x / np.maximum(n, 1e-12)[:, None, :]


_NC_CACHE = [None]


def _prepare_in_maps(f1, f2, W1, g1, b1, m1, v1, W2, g2, b2, m2, v2):
    f1 = np.asarray(f1, np.float32).reshape(S, C, HW)
    f2 = np.asarray(f2, np.float32).reshape(Q, C, HW)
    W1 = np.asarray(W1, np.float32)
    W2 = np.asarray(W2, np.float32)
    g1, b1, m1, v1 = (np.asarray(a, np.float32) for a in (g1, b1, m1, v1))
    g2, b2, m2, v2 = (np.asarray(a, np.float32) for a in (g2, b2, m2, v2))

    # host meta-learner (tiny): a1 [S, HW], a2 [Q, HW]
    a1 = _meta_learner_host(f1, W1, g1, b1, m1, v1, W2, g2, b2, m2, v2)
    a2 = _meta_learner_host(f2, W1, g1, b1, m1, v1, W2, g2, b2, m2, v2)

    f1n = _l2n(f1).astype(HDTNP)
    f2n = np.zeros((Q_PAD, C, HW), HDTNP)
    f2n[:Q] = _l2n(f2).astype(HDTNP)
    a2p = np.zeros((Q_PAD, HW), np.float32)
    a2p[:Q] = a2

    in_maps = []
    for core in range(NCORES):
        qq = [core * L + l for l in range(L)]
        a1r = np.zeros((L, S, HW), np.float32)
        a2r = np.zeros((L, HW), np.float32)
        for l, q in enumerate(qq):
            if q < Q:
                for ss in range(S):
                    i1 = (q * S + ss) // Q  # faithful torch-layout quirk
                    a1r[l, ss] = a1[i1]
                a2r[l] = a2p[q]
        in_maps.append({
            "f1n": f1n,
            "f2s": f2n[core * L:(core + 1) * L],
            "a1r": a1r.astype(HDTNP),
            "a2r": a2r.astype(HDTNP),
        })

    return in_maps


def _assemble(res):
    s1 = np.zeros((Q, S), np.float32)
    s2 = np.zeros((Q, S), np.float32)
    for core in range(NCORES):
        o = res.results[core]["out"].reshape(2, L, S)
        for l in range(L):
            q = core * L + l
            if q < Q:
                s1[q] = o[0, l]
                s2[q] = o[1, l]
    return np.stack([s1, s2])


def kernel(**inputs):
    in_maps = _prepare_in_maps(**inputs)
    if _NC_CACHE[0] is None:
        _NC_CACHE[0] = build_program()
    res = run_bass_kernel_spmd(_NC_CACHE[0], in_maps, list(range(NCORES)))
    return _assemble(res)


# revision 4
# speedup vs baseline: 1.2126x; 1.2126x over previous
"""Trainium2 Bass kernel for nn_CPSN (retrieval_knn PSM/PWG module).

Contract: kernel(**inputs) takes the FULL unsharded inputs (as produced by
setup_inputs) and returns the FULL output [2, b*q, s], distributing work
across 8 NeuronCores internally (data-parallel over the query dim q).

Algorithm per (q, s) pair (b=1, s=25, q=30, c=512, hw=361):
  O[x, y] = <f2n[:, x], f1n[:, y]>   (x = query pixel, y = support pixel)
  s21[x] = max_y O ; s12[y] = max_x O
  g1[x] = a1[argmax_y O[x, :]] ; g2[y] = a2[argmax_x O[:, y]]
  w = g1 * g2 ; out0 = mean(s12 * w) ; out1 = mean(s21 * w)

Design notes:
  - L2 normalization and the meta-learner run on host; the device receives
    pre-normalized bf16 features (bf16 matmul = 1 PE cycle/row vs fp32's 4).
  - bf16 halves SBUF/DMA: the full f1n fits resident, no ss-blocking.
  - Each PSUM tile is staged to SBUF as bf16 by the Activation engine; the
    DVE reduce_max and the fused argmax-gather
    (scalar_tensor_tensor is_ge/mult with accum) read the staged bf16 tile,
    unlocking the DVE 4x perf mode for the gather pass.
"""

import os
import sys

import numpy as np

for _p in ("/opt/trn_rl_repo", "/root/.axon_site/_ro/trn_rl_repo"):
    if os.path.isdir(_p) and _p not in sys.path:
        sys.path.insert(0, _p)

import concourse.bass as bass
import concourse.tile as tile
from concourse import bacc, library_config, mybir
from concourse.bass_utils import run_bass_kernel_spmd

# ---- problem constants (hardcoded per contract) ----
B, S, Q, C, H, W, TEMP = 1, 25, 30, 512, 19, 19, 64
HW = H * W  # 361
NCORES = 8
L = 4               # local (padded) query images per core; Q_PAD = 32
Q_PAD = NCORES * L
CCH = C // 128      # 4 contraction chunks
PCH = [(0, 128), (128, 128), (256, HW - 256)]  # pixel-dim partition chunks
GRP = 4             # O-phase ss group size (PSUM bank budget)
BN_EPS = 1e-5

F32 = mybir.dt.float32
HDT = mybir.dt.float16
AX_X = mybir.AxisListType.X
OP = mybir.AluOpType
AF = mybir.ActivationFunctionType
HDTNP = np.float16


def _col_off(l, kind, pch, ss):
    # cols2d free layout: [L][kind:4][pchunk:3][S]
    return ((l * 4 + kind) * 3 + pch) * S + ss


def build_program(variant="", repeat=1, mmdt=None):
    """Build the (SPMD-shared) single-core bass program.

    mmdt: dtype for the feature tiles / matmul inputs (default fp16).
    """
    mmdt = mmdt or HDT
    nc = bacc.Bacc(None, target_bir_lowering=False, debug=False)

    f1_d = nc.dram_tensor("f1n", [S, C, HW], mmdt, kind="ExternalInput")
    f2_d = nc.dram_tensor("f2s", [L, C, HW], mmdt, kind="ExternalInput")
    # attention rows, host-gathered per (l, ss); broadcast to 128 rows on-device
    a1r_d = nc.dram_tensor("a1r", [L, S, HW], HDT, kind="ExternalInput")
    a2r_d = nc.dram_tensor("a2r", [L, HW], HDT, kind="ExternalInput")
    out_d = nc.dram_tensor("out", [2 * L, S], F32, kind="ExternalOutput")

    with tile.TileContext(nc) as tc:
        from contextlib import ExitStack

        with ExitStack() as ctx:
            pp = ctx.enter_context(tc.tile_pool(name="pp", bufs=2, space="PSUM"))
            nsep = CCH if "sep" in variant else 1
            f1n_pool = ctx.enter_context(
                tc.tile_pool(name="f1n", bufs=S * nsep))
            f2n_pool = ctx.enter_context(
                tc.tile_pool(name="f2n", bufs=L * nsep))
            a1bc_pool = ctx.enter_context(tc.tile_pool(name="a1bc", bufs=2 * S))
            a2bc_pool = ctx.enter_context(tc.tile_pool(name="a2bc", bufs=L))
            stg_pool = ctx.enter_context(tc.tile_pool(name="stg", bufs=4))
            half_pool = ctx.enter_context(tc.tile_pool(name="half", bufs=4))
            scr_pool = ctx.enter_context(tc.tile_pool(name="scr", bufs=2))
            cols_pool = ctx.enter_context(tc.tile_pool(name="cols", bufs=1))
            cst_pool = ctx.enter_context(tc.tile_pool(name="cst", bufs=2))
            fin_pool = ctx.enter_context(tc.tile_pool(name="fin", bufs=6))

            usehalf = "nohalf" not in variant

            mcol = cst_pool.tile([128, 1], F32, tag="cst")
            nc.vector.memset(mcol[:], 1.0 / HW)

            def group_max(stg, np_, ng, maxout):
                """max over the last axis of stg[0:np_, 0:ng, :] -> maxout
                cols [np_, ng].  Plain fp16 tensor_tensor(max) runs in the
                DVE 2x perf mode while tensor_reduce is always 1x, so two
                2x halving passes (361 -> 181 -> 91; the seam overlap is
                harmless for max) ahead of the 1x reduce cut the total DVE
                cycles per group."""
                if not usehalf:
                    nc.vector.reduce_max(maxout, stg[0:np_, 0:ng, :],
                                         axis=AX_X)
                    return
                h1 = half_pool.tile([128, L, 181], HDT, name="h1", tag="h1")
                nc.vector.tensor_tensor(
                    h1[0:np_, 0:ng, :], stg[0:np_, 0:ng, 0:181],
                    stg[0:np_, 0:ng, 180:361], op=OP.max)
                h2 = half_pool.tile([128, L, 91], HDT, name="h2", tag="h2")
                nc.vector.tensor_tensor(
                    h2[0:np_, 0:ng, :], h1[0:np_, 0:ng, 0:91],
                    h1[0:np_, 0:ng, 90:181], op=OP.max)
                nc.vector.reduce_max(maxout, h2[0:np_, 0:ng, :], axis=AX_X)

            cols2d = cols_pool.tile([128, L * 4 * 3 * S], F32)
            cols12 = cols_pool.tile([128, 3 * S * L], F32)
            if variant:
                nc.vector.memset(cols2d[:], 1.0)
                nc.vector.memset(cols12[:], 1.0)

            # ---- load all features (bf16, fully resident); one DMA per
            # ---- image: [C, HW] dram -> [128, CCH, HW] sbuf (c in free dim).
            # ---- f2/attention rows first so phase-1 consumers aren't queued
            # ---- behind the 25 f1 image DMAs.
            def load_image(dram_t, img_idx, name, pool, tag):
                if "sep" in variant:
                    # baseline-style: one [128, HW] tile per c-chunk
                    ts = []
                    for c in range(CCH):
                        t = pool.tile([128, HW], mmdt, name=f"{name}_{c}",
                                      tag=tag)
                        nc.sync.dma_start(
                            t[:],
                            dram_t[img_idx, c * 128:(c + 1) * 128, :])
                        ts.append(t)
                    return ts
                t = pool.tile([128, CCH, HW], mmdt, name=name, tag=tag)
                src = bass.AP(dram_t.ap().tensor, img_idx * C * HW,
                              [[HW, 128], [128 * HW, CCH], [1, HW]])
                nc.sync.dma_start(t[:], src)
                return t

            def mm_ops(t, c, f0, fn_):
                # stationary/moving operand slice for contraction chunk c
                if "sep" in variant:
                    return t[c][:, f0:f0 + fn_]
                return t[:, c, f0:f0 + fn_]

            f2nt = [load_image(f2_d, l, f"f2n_{l}", f2n_pool, "f2n")
                    for l in range(L)]

            def bcast_row(dram_t, row_off, name, pool, tag):
                # replicate one DRAM row into all 128 partitions (stride-0
                # partition dim on the DMA source)
                t = pool.tile([128, HW], HDT, name=f"bc_{name}", tag=tag)
                src = bass.AP(dram_t.ap().tensor, row_off * HW,
                              [[0, 128], [1, HW]])
                nc.sync.dma_start(t[:], src)
                return t

            # a2 broadcast tiles (persist whole kernel)
            a2bc = [bcast_row(a2r_d, l, f"a2_{l}", a2bc_pool, "a2bc")
                    for l in range(L)]

            f1nt = [load_image(f1_d, ss, f"f1n_{ss}", f1n_pool, "f1n")
                    for ss in range(S)]

            for _rep in range(repeat):
                # ---- T phase: T[y, x] per (l, ss); weights = f1n chunks ----
                for ss in range(S):
                    for pi, (y0, yp) in enumerate(PCH):
                        psT = pp.tile([yp, L, 512], F32, name="psT", tag="ps")
                        if "nomm" not in variant:
                            for c in range(CCH):
                                for l in range(L):
                                    nc.tensor.matmul(
                                        psT[:, l, 0:HW],
                                        mm_ops(f1nt[ss], c, y0, yp),
                                        mm_ops(f2nt[l], c, 0, HW),
                                        start=(c == 0), stop=(c == CCH - 1))
                        else:
                            nc.vector.memset(psT[:, :, 0:1], 0.1)
                        if "nodve" in variant:
                            continue
                        stg = stg_pool.tile([128, L, HW], HDT, name="stgT",
                                            tag="stg")
                        nc.scalar.activation(
                            stg[0:yp, :, :], psT[:, :, 0:HW], AF.Copy)
                        o12 = (pi * S + ss) * L
                        group_max(stg, yp, L, cols12[0:yp, o12:o12 + L])
                        for l in range(L):
                            og = _col_off(l, 3, pi, ss)
                            scr = scr_pool.tile([128, HW], HDT, name="sttscr",
                                                tag="scr")
                            nc.vector.scalar_tensor_tensor(
                                scr[0:yp, :],
                                stg[0:yp, l, :],
                                cols12[0:yp, o12 + l:o12 + l + 1],
                                a2bc[l][0:yp, :],
                                op0=OP.is_ge, op1=OP.mult,
                                accum_out=cols2d[0:yp, og:og + 1])

                # ---- O phase: O[x, y] per (l, ss); weights = f2n chunks ----
                for l in range(L):
                    a1t = {}
                    for ss in range(S):
                        if "nobc" in variant:
                            a1t[ss] = a2bc[l]
                        else:
                            a1t[ss] = bcast_row(a1r_d, l * S + ss,
                                                f"a1_{l}_{ss}", a1bc_pool,
                                                "a1bc")
                    for pi, (x0, xp) in enumerate(PCH):
                        for g0 in range(0, S, GRP):
                            grp = list(range(g0, min(g0 + GRP, S)))
                            ng = len(grp)
                            psO = pp.tile([xp, L, 512], F32, name="psO",
                                          tag="ps")
                            if "nomm" not in variant:
                                for c in range(CCH):
                                    for j, ss in enumerate(grp):
                                        nc.tensor.matmul(
                                            psO[:, j, 0:HW],
                                            mm_ops(f2nt[l], c, x0, xp),
                                            mm_ops(f1nt[ss], c, 0, HW),
                                            start=(c == 0), stop=(c == CCH - 1))
                            else:
                                nc.vector.memset(psO[:, :, 0:1], 0.1)
                            if "nodve" in variant:
                                continue
                            stg = stg_pool.tile([128, L, HW], HDT,
                                                name="stgO", tag="stg")
                            nc.scalar.activation(
                                stg[0:xp, 0:ng, :],
                                psO[:, 0:ng, 0:HW], AF.Copy)
                            # s21 for the ng consecutive ss: contiguous cols
                            ob = _col_off(l, 0, pi, grp[0])
                            group_max(stg, xp, ng, cols2d[0:xp, ob:ob + ng])
                            for j, ss in enumerate(grp):
                                og = _col_off(l, 2, pi, ss)
                                scr = scr_pool.tile([128, HW], HDT,
                                                    name="sttscr", tag="scr")
                                nc.vector.scalar_tensor_tensor(
                                    scr[0:xp, :],
                                    stg[0:xp, j, :],
                                    cols2d[0:xp, ob + j:ob + j + 1],
                                    a1t[ss][0:xp, :],
                                    op0=OP.is_ge, op1=OP.mult,
                                    accum_out=cols2d[0:xp, og:og + 1])

                # ---- finals: w = g1*g2; out0 = mean(s12*w); out1 = mean(s21*w)
                for l in range(L):
                    fp1 = pp.tile([1, S], F32, tag="ps")
                    fp2 = pp.tile([1, S], F32, tag="ps")
                    for pi, (p0, pn) in enumerate(PCH):
                        g1 = cols2d[0:pn,
                                    _col_off(l, 2, pi, 0):_col_off(l, 2, pi, 0) + S]
                        g2 = cols2d[0:pn,
                                    _col_off(l, 3, pi, 0):_col_off(l, 3, pi, 0) + S]
                        s21 = cols2d[0:pn,
                                     _col_off(l, 0, pi, 0):_col_off(l, 0, pi, 0) + S]
                        c12 = cols12[0:pn, :]
                        s12 = bass.AP(c12.tensor, c12.offset + pi * S * L + l,
                                      [c12.ap[0], [L, S]])
                        wt = fin_pool.tile([128, S], F32, tag="fin")
                        v1 = fin_pool.tile([128, S], F32, tag="fin")
                        v2 = fin_pool.tile([128, S], F32, tag="fin")
                        nc.vector.tensor_mul(wt[0:pn, :], g1, g2)
                        nc.vector.tensor_mul(v1[0:pn, :], s12, wt[0:pn, :])
                        nc.vector.tensor_mul(v2[0:pn, :], s21, wt[0:pn, :])
                        nc.tensor.matmul(fp1[:, :], mcol[0:pn, 0:1],
                                         v1[0:pn, :],
                                         start=(pi == 0), stop=(pi == 2))
                        nc.tensor.matmul(fp2[:, :], mcol[0:pn, 0:1],
                                         v2[0:pn, :],
                                         start=(pi == 0), stop=(pi == 2))
                    st1 = fin_pool.tile([1, S], F32, name=f"st1_{l}",
                                        tag="finst")
                    st2 = fin_pool.tile([1, S], F32, name=f"st2_{l}",
                                        tag="finst")
                    nc.scalar.activation(st1[:], fp1[0:1, :], AF.Copy)
                    nc.scalar.activation(st2[:], fp2[0:1, :], AF.Copy)
                    nc.sync.dma_start(out_d[l:l + 1, :], st1[0:1, :])
                    nc.sync.dma_start(out_d[L + l:L + l + 1, :], st2[0:1, :])

    nc.finalize()
    return nc


def _meta_learner_host(x, W1, g1, b1, m1, v1, W2, g2, b2, m2, v2):
    """x: [N, C, HW] -> [N, HW]  (two 1x1 convs + eval BN + ReLU on host)."""
    inv1 = g1 / np.sqrt(v1 + BN_EPS)
    bias1 = b1 - m1 * inv1
    y = np.einsum("tc,ncp->ntp", W1, x, dtype=np.float32)
    y = np.maximum(y * inv1[None, :, None] + bias1[None, :, None], 0.0)
    inv2 = g2 / np.sqrt(v2 + BN_EPS)
    bias2 = b2 - m2 * inv2
    z = np.einsum("ot,ntp->nop", W2, y, dtype=np.float32)
    z = np.maximum(z * inv2[None, :, None] + bias2[None, :, None], 0.0)
    return z[:, 0, :]


def _l2n(x):
    n = np.sqrt(np.einsum("ncp,ncp->np", x, x, dtype=np.float32))
    return x / np.maximum(n, 1e-12)[:, None, :]


_NC_CACHE = [None]


def _prepare_in_maps(f1, f2, W1, g1, b1, m1, v1, W2, g2, b2, m2, v2):
    f1 = np.asarray(f1, np.float32).reshape(S, C, HW)
    f2 = np.asarray(f2, np.float32).reshape(Q, C, HW)
    W1 = np.asarray(W1, np.float32)
    W2 = np.asarray(W2, np.float32)
    g1, b1, m1, v1 = (np.asarray(a, np.float32) for a in (g1, b1, m1, v1))
    g2, b2, m2, v2 = (np.asarray(a, np.float32) for a in (g2, b2, m2, v2))

    # host meta-learner (tiny): a1 [S, HW], a2 [Q, HW]
    a1 = _meta_learner_host(f1, W1, g1, b1, m1, v1, W2, g2, b2, m2, v2)
    a2 = _meta_learner_host(f2, W1, g1, b1, m1, v1, W2, g2, b2, m2, v2)

    f1n = _l2n(f1).astype(HDTNP)
    f2n = np.zeros((Q_PAD, C, HW), HDTNP)
    f2n[:Q] = _l2n(f2).astype(HDTNP)
    a2p = np.zeros((Q_PAD, HW), np.float32)
    a2p[:Q] = a2

    in_maps = []
    for core in range(NCORES):
        qq = [core * L + l for l in range(L)]
        a1r = np.zeros((L, S, HW), np.float32)
        a2r = np.zeros((L, HW), np.float32)
        for l, q in enumerate(qq):
            if q < Q:
                for ss in range(S):
                    i1 = (q * S + ss) // Q  # faithful torch-layout quirk
                    a1r[l, ss] = a1[i1]
                a2r[l] = a2p[q]
        in_maps.append({
            "f1n": f1n,
            "f2s": f2n[core * L:(core + 1) * L],
            "a1r": a1r.astype(HDTNP),
            "a2r": a2r.astype(HDTNP),
        })

    return in_maps


def _assemble(res):
    s1 = np.zeros((Q, S), np.float32)
    s2 = np.zeros((Q, S), np.float32)
    for core in range(NCORES):
        o = res.results[core]["out"].reshape(2, L, S)
        for l in range(L):
            q = core * L + l
            if q < Q:
                s1[q] = o[0, l]
                s2[q] = o[1, l]
    return np.stack([s1, s2])


def kernel(**inputs):
    in_maps = _prepare_in_maps(**inputs)
    if _NC_CACHE[0] is None:
        _NC_CACHE[0] = build_program()
    res = run_bass_kernel_spmd(_NC_CACHE[0], in_maps, list(range(NCORES)))
    return _assemble(res)


# revision 5
# speedup vs baseline: 1.2918x; 1.0653x over previous
"""Trainium2 Bass kernel for nn_CPSN (retrieval_knn PSM/PWG module).

Contract: kernel(**inputs) takes the FULL unsharded inputs (as produced by
setup_inputs) and returns the FULL output [2, b*q, s], distributing work
across 8 NeuronCores internally (data-parallel over the query dim q).

Algorithm per (q, s) pair (b=1, s=25, q=30, c=512, hw=361):
  O[x, y] = <f2n[:, x], f1n[:, y]>   (x = query pixel, y = support pixel)
  s21[x] = max_y O ; s12[y] = max_x O
  g1[x] = a1[argmax_y O[x, :]] ; g2[y] = a2[argmax_x O[:, y]]
  w = g1 * g2 ; out0 = mean(s12 * w) ; out1 = mean(s21 * w)

Design notes:
  - L2 normalization and the meta-learner run on host; the device receives
    pre-normalized bf16 features (bf16 matmul = 1 PE cycle/row vs fp32's 4).
  - bf16 halves SBUF/DMA: the full f1n fits resident, no ss-blocking.
  - Each PSUM tile is staged to SBUF as bf16 by the Activation engine; the
    DVE reduce_max and the fused argmax-gather
    (scalar_tensor_tensor is_ge/mult with accum) read the staged bf16 tile,
    unlocking the DVE 4x perf mode for the gather pass.
"""

import os
import sys

import numpy as np

for _p in ("/opt/trn_rl_repo", "/root/.axon_site/_ro/trn_rl_repo"):
    if os.path.isdir(_p) and _p not in sys.path:
        sys.path.insert(0, _p)

import concourse.bass as bass
import concourse.tile as tile
from concourse import bacc, library_config, mybir
from concourse.bass_utils import run_bass_kernel_spmd

# ---- problem constants (hardcoded per contract) ----
B, S, Q, C, H, W, TEMP = 1, 25, 30, 512, 19, 19, 64
HW = H * W  # 361
NCORES = 8
L = 4               # local (padded) query images per core; Q_PAD = 32
Q_PAD = NCORES * L
CCH = C // 128      # 4 contraction chunks
PCH = [(0, 128), (128, 128), (256, HW - 256)]  # pixel-dim partition chunks
GRP = 4             # O-phase ss group size (PSUM bank budget)
BN_EPS = 1e-5

F32 = mybir.dt.float32
HDT = mybir.dt.float16
AX_X = mybir.AxisListType.X
OP = mybir.AluOpType
AF = mybir.ActivationFunctionType
HDTNP = np.float16


def _col_off(l, kind, pch, ss):
    # cols2d free layout: [L][kind:4][pchunk:3][S]
    return ((l * 4 + kind) * 3 + pch) * S + ss


def build_program(variant="", repeat=1, mmdt=None):
    """Build the (SPMD-shared) single-core bass program.

    mmdt: dtype for the feature tiles / matmul inputs (default fp16).
    """
    mmdt = mmdt or HDT
    nc = bacc.Bacc(None, target_bir_lowering=False, debug=False)

    f1_d = nc.dram_tensor("f1n", [S, C, HW], mmdt, kind="ExternalInput")
    f2_d = nc.dram_tensor("f2s", [L, C, HW], mmdt, kind="ExternalInput")
    # attention rows, host-gathered per (l, ss); broadcast to 128 rows on-device
    a1r_d = nc.dram_tensor("a1r", [L, S, HW], HDT, kind="ExternalInput")
    a2r_d = nc.dram_tensor("a2r", [L, HW], HDT, kind="ExternalInput")
    out_d = nc.dram_tensor("out", [2 * L, S], F32, kind="ExternalOutput")

    with tile.TileContext(nc) as tc:
        from contextlib import ExitStack

        with ExitStack() as ctx:
            pp = ctx.enter_context(tc.tile_pool(name="pp", bufs=2, space="PSUM"))
            nsep = CCH if "sep" in variant else 1
            f1n_pool = ctx.enter_context(
                tc.tile_pool(name="f1n", bufs=S * nsep))
            f2n_pool = ctx.enter_context(
                tc.tile_pool(name="f2n", bufs=L * nsep))
            a1bc_pool = ctx.enter_context(tc.tile_pool(name="a1bc", bufs=2 * S))
            a2bc_pool = ctx.enter_context(tc.tile_pool(name="a2bc", bufs=L))
            stg_pool = ctx.enter_context(tc.tile_pool(name="stg", bufs=4))
            half_pool = ctx.enter_context(tc.tile_pool(name="half", bufs=4))
            scr_pool = ctx.enter_context(tc.tile_pool(name="scr", bufs=2))
            cols_pool = ctx.enter_context(tc.tile_pool(name="cols", bufs=1))
            cst_pool = ctx.enter_context(tc.tile_pool(name="cst", bufs=2))
            fin_pool = ctx.enter_context(tc.tile_pool(name="fin", bufs=6))

            usehalf = "nohalf" not in variant

            mcol = cst_pool.tile([128, 1], F32, tag="cst")
            nc.vector.memset(mcol[:], 1.0 / HW)

            def group_max(stg, np_, ng, maxout):
                """max over the last axis of stg[0:np_, 0:ng, :] -> maxout
                cols [np_, ng].  Plain fp16 tensor_tensor(max) runs in the
                DVE 2x perf mode while tensor_reduce is always 1x, so two
                2x halving passes (361 -> 181 -> 91; the seam overlap is
                harmless for max) ahead of the 1x reduce cut the total DVE
                cycles per group."""
                if not usehalf:
                    nc.vector.reduce_max(maxout, stg[0:np_, 0:ng, :],
                                         axis=AX_X)
                    return
                h1 = half_pool.tile([128, L, 181], HDT, name="h1", tag="h1")
                nc.vector.tensor_tensor(
                    h1[0:np_, 0:ng, :], stg[0:np_, 0:ng, 0:181],
                    stg[0:np_, 0:ng, 180:361], op=OP.max)
                h2 = half_pool.tile([128, L, 91], HDT, name="h2", tag="h2")
                nc.vector.tensor_tensor(
                    h2[0:np_, 0:ng, :], h1[0:np_, 0:ng, 0:91],
                    h1[0:np_, 0:ng, 90:181], op=OP.max)
                nc.vector.reduce_max(maxout, h2[0:np_, 0:ng, :], axis=AX_X)

            cols2d = cols_pool.tile([128, L * 4 * 3 * S], F32)
            cols12 = cols_pool.tile([128, 3 * S * L], F32)
            if variant:
                nc.vector.memset(cols2d[:], 1.0)
                nc.vector.memset(cols12[:], 1.0)

            # ---- load all features (bf16, fully resident); one DMA per
            # ---- image: [C, HW] dram -> [128, CCH, HW] sbuf (c in free dim).
            # ---- f2/attention rows first so phase-1 consumers aren't queued
            # ---- behind the 25 f1 image DMAs.
            def load_image(dram_t, img_idx, name, pool, tag):
                if "sep" in variant:
                    # baseline-style: one [128, HW] tile per c-chunk
                    ts = []
                    for c in range(CCH):
                        t = pool.tile([128, HW], mmdt, name=f"{name}_{c}",
                                      tag=tag)
                        nc.sync.dma_start(
                            t[:],
                            dram_t[img_idx, c * 128:(c + 1) * 128, :])
                        ts.append(t)
                    return ts
                t = pool.tile([128, CCH, HW], mmdt, name=name, tag=tag)
                src = bass.AP(dram_t.ap().tensor, img_idx * C * HW,
                              [[HW, 128], [128 * HW, CCH], [1, HW]])
                nc.sync.dma_start(t[:], src)
                return t

            def mm_ops(t, c, f0, fn_):
                # stationary/moving operand slice for contraction chunk c
                if "sep" in variant:
                    return t[c][:, f0:f0 + fn_]
                return t[:, c, f0:f0 + fn_]

            f2nt = [load_image(f2_d, l, f"f2n_{l}", f2n_pool, "f2n")
                    for l in range(L)]

            def bcast_row(dram_t, row_off, name, pool, tag, nrows=1):
                # replicate nrows consecutive DRAM rows into all 128
                # partitions (stride-0 partition dim on the DMA source)
                t = pool.tile([128, nrows, HW], HDT, name=f"bc_{name}",
                              tag=tag)
                src = bass.AP(dram_t.ap().tensor, row_off * HW,
                              [[0, 128], [1, nrows * HW]])
                nc.sync.dma_start(t[:], src)
                return t

            # a2 broadcast tiles (persist whole kernel)
            a2bc = [bcast_row(a2r_d, l, f"a2_{l}", a2bc_pool, "a2bc")
                    for l in range(L)]

            def make_a1t(l):
                """broadcast the 25 a1 rows for query slot l, two per DMA"""
                a1t = {}
                for s0 in range(0, S, 2):
                    nr = min(2, S - s0)
                    t = bcast_row(a1r_d, l * S + s0, f"a1_{l}_{s0}",
                                  a1bc_pool, "a1bc", nrows=nr)
                    for j in range(nr):
                        a1t[s0 + j] = (t, j)
                return a1t

            f1nt = [load_image(f1_d, ss, f"f1n_{ss}", f1n_pool, "f1n")
                    for ss in range(S)]

            for _rep in range(repeat):
                # ---- T phase: T[y, x] per (l, ss); weights = f1n chunks ----
                for ss in range(S):
                    for pi, (y0, yp) in enumerate(PCH):
                        psT = pp.tile([yp, L, 512], F32, name="psT", tag="ps")
                        if "nomm" not in variant:
                            for c in range(CCH):
                                for l in range(L):
                                    nc.tensor.matmul(
                                        psT[:, l, 0:HW],
                                        mm_ops(f1nt[ss], c, y0, yp),
                                        mm_ops(f2nt[l], c, 0, HW),
                                        start=(c == 0), stop=(c == CCH - 1))
                        else:
                            nc.vector.memset(psT[:, :, 0:1], 0.1)
                        if "nodve" in variant:
                            continue
                        stg = stg_pool.tile([128, L, HW], HDT, name="stgT",
                                            tag="stg")
                        nc.scalar.activation(
                            stg[0:yp, :, :], psT[:, :, 0:HW], AF.Copy)
                        o12 = (pi * S + ss) * L
                        group_max(stg, yp, L, cols12[0:yp, o12:o12 + L])
                        for l in range(L):
                            og = _col_off(l, 3, pi, ss)
                            scr = scr_pool.tile([128, HW], HDT, name="sttscr",
                                                tag="scr")
                            nc.vector.scalar_tensor_tensor(
                                scr[0:yp, :],
                                stg[0:yp, l, :],
                                cols12[0:yp, o12 + l:o12 + l + 1],
                                a2bc[l][0:yp, 0, :],
                                op0=OP.is_ge, op1=OP.mult,
                                accum_out=cols2d[0:yp, og:og + 1])

                # ---- O phase: O[x, y] per (l, ss); weights = f2n chunks ----
                for l in range(L):
                    if "nobc" in variant:
                        a1t = {ss: (a2bc[l], 0) for ss in range(S)}
                    else:
                        a1t = make_a1t(l)
                    for pi, (x0, xp) in enumerate(PCH):
                        for g0 in range(0, S, GRP):
                            grp = list(range(g0, min(g0 + GRP, S)))
                            ng = len(grp)
                            psO = pp.tile([xp, L, 512], F32, name="psO",
                                          tag="ps")
                            if "nomm" not in variant:
                                for c in range(CCH):
                                    for j, ss in enumerate(grp):
                                        nc.tensor.matmul(
                                            psO[:, j, 0:HW],
                                            mm_ops(f2nt[l], c, x0, xp),
                                            mm_ops(f1nt[ss], c, 0, HW),
                                            start=(c == 0), stop=(c == CCH - 1))
                            else:
                                nc.vector.memset(psO[:, :, 0:1], 0.1)
                            if "nodve" in variant:
                                continue
                            stg = stg_pool.tile([128, L, HW], HDT,
                                                name="stgO", tag="stg")
                            nc.scalar.activation(
                                stg[0:xp, 0:ng, :],
                                psO[:, 0:ng, 0:HW], AF.Copy)
                            # s21 for the ng consecutive ss: contiguous cols
                            ob = _col_off(l, 0, pi, grp[0])
                            group_max(stg, xp, ng, cols2d[0:xp, ob:ob + ng])
                            for j, ss in enumerate(grp):
                                og = _col_off(l, 2, pi, ss)
                                scr = scr_pool.tile([128, HW], HDT,
                                                    name="sttscr", tag="scr")
                                nc.vector.scalar_tensor_tensor(
                                    scr[0:xp, :],
                                    stg[0:xp, j, :],
                                    cols2d[0:xp, ob + j:ob + j + 1],
                                    a1t[ss][0][0:xp, a1t[ss][1], :],
                                    op0=OP.is_ge, op1=OP.mult,
                                    accum_out=cols2d[0:xp, og:og + 1])

                # ---- finals: w = g1*g2; out0 = mean(s12*w); out1 = mean(s21*w)
                for l in range(L):
                    fp1 = pp.tile([1, S], F32, tag="ps")
                    fp2 = pp.tile([1, S], F32, tag="ps")
                    for pi, (p0, pn) in enumerate(PCH):
                        g1 = cols2d[0:pn,
                                    _col_off(l, 2, pi, 0):_col_off(l, 2, pi, 0) + S]
                        g2 = cols2d[0:pn,
                                    _col_off(l, 3, pi, 0):_col_off(l, 3, pi, 0) + S]
                        s21 = cols2d[0:pn,
                                     _col_off(l, 0, pi, 0):_col_off(l, 0, pi, 0) + S]
                        c12 = cols12[0:pn, :]
                        s12 = bass.AP(c12.tensor, c12.offset + pi * S * L + l,
                                      [c12.ap[0], [L, S]])
                        wt = fin_pool.tile([128, S], F32, tag="fin")
                        v1 = fin_pool.tile([128, S], F32, tag="fin")
                        v2 = fin_pool.tile([128, S], F32, tag="fin")
                        nc.vector.tensor_mul(wt[0:pn, :], g1, g2)
                        nc.vector.tensor_mul(v1[0:pn, :], s12, wt[0:pn, :])
                        nc.vector.tensor_mul(v2[0:pn, :], s21, wt[0:pn, :])
                        nc.tensor.matmul(fp1[:, :], mcol[0:pn, 0:1],
                                         v1[0:pn, :],
                                         start=(pi == 0), stop=(pi == 2))
                        nc.tensor.matmul(fp2[:, :], mcol[0:pn, 0:1],
                                         v2[0:pn, :],
                                         start=(pi == 0), stop=(pi == 2))
                    st1 = fin_pool.tile([1, S], F32, name=f"st1_{l}",
                                        tag="finst")
                    st2 = fin_pool.tile([1, S], F32, name=f"st2_{l}",
                                        tag="finst")
                    nc.scalar.activation(st1[:], fp1[0:1, :], AF.Copy)
                    nc.scalar.activation(st2[:], fp2[0:1, :], AF.Copy)
                    nc.sync.dma_start(out_d[l:l + 1, :], st1[0:1, :])
                    nc.sync.dma_start(out_d[L + l:L + l + 1, :], st2[0:1, :])

    nc.finalize()
    return nc


def _meta_learner_host(x, W1, g1, b1, m1, v1, W2, g2, b2, m2, v2):
    """x: [N, C, HW] -> [N, HW]  (two 1x1 convs + eval BN + ReLU on host)."""
    inv1 = g1 / np.sqrt(v1 + BN_EPS)
    bias1 = b1 - m1 * inv1
    y = np.einsum("tc,ncp->ntp", W1, x, dtype=np.float32)
    y = np.maximum(y * inv1[None, :, None] + bias1[None, :, None], 0.0)
    inv2 = g2 / np.sqrt(v2 + BN_EPS)
    bias2 = b2 - m2 * inv2
    z = np.einsum("ot,ntp->nop", W2, y, dtype=np.float32)
    z = np.maximum(z * inv2[None, :, None] + bias2[None, :, None], 0.0)
    return z[:, 0, :]


def _l2n(x):
    n = np.sqrt(np.einsum("ncp,ncp->np", x, x, dtype=np.float32))
    return x / np.maximum(n, 1e-12)[:, None, :]


_NC_CACHE = [None]


def _prepare_in_maps(f1, f2, W1, g1, b1, m1, v1, W2, g2, b2, m2, v2):
    f1 = np.asarray(f1, np.float32).reshape(S, C, HW)
    f2 = np.asarray(f2, np.float32).reshape(Q, C, HW)
    W1 = np.asarray(W1, np.float32)
    W2 = np.asarray(W2, np.float32)
    g1, b1, m1, v1 = (np.asarray(a, np.float32) for a in (g1, b1, m1, v1))
    g2, b2, m2, v2 = (np.asarray(a, np.float32) for a in (g2, b2, m2, v2))

    # host meta-learner (tiny): a1 [S, HW], a2 [Q, HW]
    a1 = _meta_learner_host(f1, W1, g1, b1, m1, v1, W2, g2, b2, m2, v2)
    a2 = _meta_learner_host(f2, W1, g1, b1, m1, v1, W2, g2, b2, m2, v2)

    f1n = _l2n(f1).astype(HDTNP)
    f2n = np.zeros((Q_PAD, C, HW), HDTNP)
    f2n[:Q] = _l2n(f2).astype(HDTNP)
    a2p = np.zeros((Q_PAD, HW), np.float32)
    a2p[:Q] = a2

    in_maps = []
    for core in range(NCORES):
        qq = [core * L + l for l in range(L)]
        a1r = np.zeros((L, S, HW), np.float32)
        a2r = np.zeros((L, HW), np.float32)
        for l, q in enumerate(qq):
            if q < Q:
                for ss in range(S):
                    i1 = (q * S + ss) // Q  # faithful torch-layout quirk
                    a1r[l, ss] = a1[i1]
                a2r[l] = a2p[q]
        in_maps.append({
            "f1n": f1n,
            "f2s": f2n[core * L:(core + 1) * L],
            "a1r": a1r.astype(HDTNP),
            "a2r": a2r.astype(HDTNP),
        })

    return in_maps


def _assemble(res):
    s1 = np.zeros((Q, S), np.float32)
    s2 = np.zeros((Q, S), np.float32)
    for core in range(NCORES):
        o = res.results[core]["out"].reshape(2, L, S)
        for l in range(L):
            q = core * L + l
            if q < Q:
                s1[q] = o[0, l]
                s2[q] = o[1, l]
    return np.stack([s1, s2])


def kernel(**inputs):
    in_maps = _prepare_in_maps(**inputs)
    if _NC_CACHE[0] is None:
        _NC_CACHE[0] = build_program()
    res = run_bass_kernel_spmd(_NC_CACHE[0], in_maps, list(range(NCORES)))
    return _assemble(res)
